# revision 1
# baseline (speedup 1.0000x reference)
"""Trainium2 Bass kernel for nn_AttentionHead (B=4, S=4096, D_IN=1024, DK=DV=64).

Sharding: 8 cores = batch(4) x query-half(2). Each core computes attention for
its 2048 query rows against the full 4096-key sequence of its batch. No
collectives.

Per-core algorithm (all matmul compute in bf16, f32 accumulation):
  1. Load raw q/k/v with a 32x32 block-swizzle cast-DMA (f32 DRAM -> bf16 SBUF),
     then DVE stream-transpose to get x^T tiles [128 d_in, seq] in SBUF.
  2. Projections with W stationary: qT = Wq^T q^T [64, 2048], kT [64, 4096],
     vT [64, 4096]; bias added on PSUM eviction (per-partition scalar).
  3. v1 = PE-transpose of vT -> v natural [kv, 64] with a ones column appended
     (column 64), so the PV matmul also accumulates the softmax denominator.
  4. scoresT[kv, q] = kT_tile^T qT (K=64), exp via ScalarE (scale=1/8) into
     bf16 expT tiles, PV: outT[65, 512] += v1_chunk^T expT (K=128 kv).
  5. Finalize: PE-transpose outT -> [128 q, 65], reciprocal of col 64,
     per-partition scale, DMA out f32.
"""
import os
import numpy as np

import concourse.bass as bass
import concourse.mybir as mybir
import concourse.tile as tile
from concourse import bacc
from concourse.bass_utils import run_bass_kernel_spmd
from concourse.masks import make_identity

F32 = mybir.dt.float32
BF16 = mybir.dt.bfloat16
AX = mybir.AxisListType.X
EXP = mybir.ActivationFunctionType.Exp

B, S, D_IN, DK, DV = 4, 4096, 1024, 64, 64
SQ = S // 2            # 2048 query rows per core
NCH = D_IN // 128      # 8 d_in chunks
NKV = S // 128         # 32 kv tiles
NQB = SQ // 512        # 4 query blocks of 512
PASS = 2048            # seq rows per load pass

_NC_CACHE = {}


def _load_pass(nc, Ap, Bp, x_ext, s0, ext_rows):
    """Swizzle cast-load + stream-transpose one pass of PASS seq rows.

    Returns list of 8 bf16 tiles Bs[c] = x[s0:s0+PASS, 128c:128c+128]^T with
    layout [128 d, PASS seq].
    """
    Bs = []
    for c in range(NCH):
        d0 = 128 * c
        A = Ap.tile([128, PASS], BF16, tag="A")
        for db in range(4):
            xin = x_ext[s0 : s0 + PASS, d0 + 32 * db : d0 + 32 * db + 32].rearrange(
                "(sb i) j -> i sb j", i=32
            )
            nc.gpsimd.dma_start(out=A[32 * db : 32 * db + 32, :], in_=xin)
        Bt = Bp.tile([128, PASS], BF16, tag="B")
        nc.vector.transpose(out=Bt[:, :], in_=A[:, :])
        Bs.append(Bt)
    return Bs


def _project(nc, pp, Bs, W, bias_t, outT, col0):
    """outT[:, col0:col0+PASS] (bf16 [64, *]) = W^T x^T + bias, accumulating
    over the 8 d_in chunks in PSUM per 512-block."""
    for sb in range(PASS // 512):
        ps = pp.tile([64, 512], F32, tag="pp")
        for c in range(NCH):
            nc.tensor.matmul(
                ps[:, :],
                W[:, c, :],
                Bs[c][:, 512 * sb : 512 * (sb + 1)],
                start=(c == 0),
                stop=(c == NCH - 1),
            )
        nc.vector.tensor_scalar_add(
            outT[:, col0 + 512 * sb : col0 + 512 * (sb + 1)], ps[:, :], bias_t[:, :]
        )


def build_attention_nc():
    nc = bacc.Bacc()

    q_ext = nc.declare_dram_parameter("q", [SQ, D_IN], F32, isOutput=False)
    k_ext = nc.declare_dram_parameter("k", [S, D_IN], F32, isOutput=False)
    v_ext = nc.declare_dram_parameter("v", [S, D_IN], F32, isOutput=False)
    wq_ext = nc.declare_dram_parameter("wq", [D_IN, DK], F32, isOutput=False)
    wk_ext = nc.declare_dram_parameter("wk", [D_IN, DK], F32, isOutput=False)
    wv_ext = nc.declare_dram_parameter("wv", [D_IN, DV], F32, isOutput=False)
    bq_ext = nc.declare_dram_parameter("bq", [DK], F32, isOutput=False)
    bk_ext = nc.declare_dram_parameter("bk", [DK], F32, isOutput=False)
    bv_ext = nc.declare_dram_parameter("bv", [DV], F32, isOutput=False)
    out_ext = nc.declare_dram_parameter("out", [SQ, DV], F32, isOutput=True)

    with tile.TileContext(nc) as tc:
        with (
            tc.tile_pool(name="single", bufs=1) as sg,
            tc.tile_pool(name="Ap", bufs=3) as Ap,
            tc.tile_pool(name="Bp", bufs=10) as Bp,
            tc.tile_pool(name="expp", bufs=4) as expp,
            tc.tile_pool(name="fin", bufs=2) as fin,
            tc.tile_pool(name="pp", bufs=2, space="PSUM") as pp,
            tc.tile_pool(name="sc", bufs=2, space="PSUM") as sc,
            tc.tile_pool(name="ot", bufs=4, space="PSUM") as ot,
        ):
            # ---- constants
            ident_b = sg.tile([128, 128], BF16)
            make_identity(nc, ident_b[:, :])
            ident_f = sg.tile([128, 128], F32)
            make_identity(nc, ident_f[:, :])

            # weights -> bf16 [128, 8, 64] (cast during DMA)
            Wq = sg.tile([128, NCH, DK], BF16)
            Wk = sg.tile([128, NCH, DK], BF16)
            Wv = sg.tile([128, NCH, DV], BF16)
            for W, ext in ((Wq, wq_ext), (Wk, wk_ext), (Wv, wv_ext)):
                nc.gpsimd.dma_start(
                    out=W[:, :, :], in_=ext.rearrange("(c p) n -> p c n", p=128)
                )
            bq_t = sg.tile([64, 1], F32)
            bk_t = sg.tile([64, 1], F32)
            bv_t = sg.tile([64, 1], F32)
            for bt, ext in ((bq_t, bq_ext), (bk_t, bk_ext), (bv_t, bv_ext)):
                nc.sync.dma_start(out=bt[:, :], in_=ext[:].unsqueeze(-1))

            # projected tensors (bf16)
            qT = sg.tile([64, SQ], BF16)    # [dk, q]
            kT = sg.tile([64, S], BF16)     # [dk, kv]
            vT = sg.tile([64, S], BF16)     # [dv, kv]
            v1 = sg.tile([128, NKV, DV + 1], BF16)  # v natural + ones col
            nc.vector.memset(v1[:, :, DV : DV + 1], 1.0)

            # prime PE's observed clock with the gpsimd tick (identity)
            prime_ps = pp.tile([128, 128], BF16, tag="pp")
            nc.tensor.transpose(prime_ps[:, :], ident_b[:, :], ident_b[:, :])

            # outT accumulators, one per query block [65, 512] f32
            otps = [ot.tile([DV + 1, 512], F32, tag="ot", name=f"otps{i}") for i in range(NQB)]

            def v_flip(c):
                # vT[:, 128c:128c+128] -> v1[:, c, :64]
                ps = pp.tile([128, DV], BF16, tag="pp")
                nc.tensor.transpose(
                    ps[:, :], vT[:, 128 * c : 128 * (c + 1)], ident_b[0:64, 0:64]
                )
                nc.scalar.copy(v1[:, c, 0:DV], ps[:, :])

            def phase_b(c):
                # scoresT + exp + PV for kv chunk c against all query blocks
                for qb in range(NQB):
                    sps = sc.tile([128, 512], F32, tag="sc")
                    nc.tensor.matmul(
                        sps[:, :],
                        kT[:, 128 * c : 128 * (c + 1)],
                        qT[:, 512 * qb : 512 * (qb + 1)],
                        start=True,
                        stop=True,
                    )
                    ex = expp.tile([128, 512], BF16, tag="ex")
                    nc.scalar.activation(
                        out=ex[:, :], in_=sps[:, :], func=EXP, scale=0.125
                    )
                    nc.tensor.matmul(
                        otps[qb][:, :],
                        v1[:, c, :],
                        ex[:, :],
                        start=(c == 0),
                        stop=(c == NKV - 1),
                    )

            # ---- phase A part 1: q, k half 1, v half 1
            Bs = _load_pass(nc, Ap, Bp, q_ext, 0, SQ)
            _project(nc, pp, Bs, Wq, bq_t, qT, 0)
            Bs = _load_pass(nc, Ap, Bp, k_ext, 0, S)
            _project(nc, pp, Bs, Wk, bk_t, kT, 0)
            Bs = _load_pass(nc, Ap, Bp, v_ext, 0, S)
            _project(nc, pp, Bs, Wv, bv_t, vT, 0)
            for c in range(NKV // 2):
                v_flip(c)

            # ---- phase B half 1 (kv chunks 0..15)
            for c in range(NKV // 2):
                phase_b(c)

            # ---- phase A part 2: k half 2, v half 2
            Bs = _load_pass(nc, Ap, Bp, k_ext, PASS, S)
            _project(nc, pp, Bs, Wk, bk_t, kT, PASS)
            Bs = _load_pass(nc, Ap, Bp, v_ext, PASS, S)
            _project(nc, pp, Bs, Wv, bv_t, vT, PASS)
            for c in range(NKV // 2, NKV):
                v_flip(c)

            # ---- phase B half 2
            for c in range(NKV // 2, NKV):
                phase_b(c)

            # ---- finalize: normalize + transpose back + store
            for qb in range(NQB):
                o_sb = fin.tile([DV + 1, 512], F32, tag="osb")
                nc.vector.tensor_copy(o_sb[:, :], otps[qb][:, :])
                for t in range(4):
                    tp = pp.tile([128, DV + 1], F32, tag="pp")
                    nc.tensor.transpose(
                        tp[:, :],
                        o_sb[:, 128 * t : 128 * (t + 1)],
                        ident_f[0 : DV + 1, 0 : DV + 1],
                    )
                    rec = fin.tile([128, 1], F32, tag="rec")
                    nc.vector.reciprocal(rec[:, :], tp[:, DV : DV + 1])
                    o_f = fin.tile([128, DV], F32, tag="of")
                    nc.vector.tensor_scalar_mul(o_f[:, :], tp[:, 0:DV], rec[:, :])
                    nc.scalar.dma_start(
                        out=out_ext[512 * qb + 128 * t : 512 * qb + 128 * (t + 1), :],
                        in_=o_f[:, :],
                    )

    nc.compile()
    return nc


def _get_nc():
    if "nc" not in _NC_CACHE:
        _NC_CACHE["nc"] = build_attention_nc()
    return _NC_CACHE["nc"]


def kernel(query, key, value, Wq, bq, Wk, bk, Wv, bv):
    query = np.asarray(query, dtype=np.float32)
    key = np.asarray(key, dtype=np.float32)
    value = np.asarray(value, dtype=np.float32)
    wq = np.ascontiguousarray(np.asarray(Wq, np.float32))
    wk = np.ascontiguousarray(np.asarray(Wk, np.float32))
    wv = np.ascontiguousarray(np.asarray(Wv, np.float32))
    bq_ = np.ascontiguousarray(np.asarray(bq, np.float32))
    bk_ = np.ascontiguousarray(np.asarray(bk, np.float32))
    bv_ = np.ascontiguousarray(np.asarray(bv, np.float32))

    in_maps = []
    for b in range(B):
        for h in range(2):
            in_maps.append(
                {
                    "q": np.ascontiguousarray(query[b, h * SQ : (h + 1) * SQ]),
                    "k": np.ascontiguousarray(key[b]),
                    "v": np.ascontiguousarray(value[b]),
                    "wq": wq, "wk": wk, "wv": wv,
                    "bq": bq_, "bk": bk_, "bv": bv_,
                }
            )

    nc = _get_nc()
    trace = bool(int(os.environ.get("BASS_KERNEL_TRACE", "0")))
    res = run_bass_kernel_spmd(nc, in_maps, core_ids=list(range(8)), trace=trace)
    _NC_CACHE["last_results"] = res

    out = np.empty((B, S, DV), np.float32)
    for b in range(B):
        for h in range(2):
            out[b, h * SQ : (h + 1) * SQ] = res.results[2 * b + h]["out"]
    return out



# revision 3
# speedup vs baseline: 2.6130x; 2.6130x over previous
"""Trainium2 Bass kernel for nn_AttentionHead (B=4, S=4096, D_IN=1024, DK=DV=64).

Sharding: 8 cores = batch(4) x query-half(2). Each core computes attention for
its 2048 query rows against the full 4096-key sequence of its batch.

Host prep: inputs are cast to bf16 and transposed to [D_IN, seq] on the host,
so the device does plain contiguous HWDGE loads (no swizzle DMA, no on-chip
stream transpose, half the HBM bytes).

Per-core device pipeline (flash-style streaming over kv granules of 4 chunks):
  1. Loads: granule tiles [128, 8, cols] bf16 via nc.sync DMA, double buffered.
  2. Projections with W stationary, W column-duplicated so PSUM rows 64-127
     hold a copy: one [128, 512] eviction (bias add) writes both the base and
     the high-partition copy used for PE row-tile packing.
  3. Scores: row-tiled pairs — chunk 2c on PE rows 0-63, chunk 2c+1 on rows
     64-127, concurrent, N=512 each, into a [128, 1024] PSUM tile.
  4. Exp on ScalarE in N=1024 blocks (f32 PSUM -> bf16 SBUF), double buffered.
  5. Softmax denominator: running per-partition sums dacc[qb] += ex on
     DVE (qb 0/1) and GpSimd (qb 2/3); final 128-partition reduce via tiny
     ones-matmuls at the end.
  6. PV: col-tiled pairs — qb pair (a,b) share one PSUM bank, M=64 each at
     output partitions 0-63 / 64-127, concurrent, accumulated over all 32
     kv chunks.
  7. Finalize: transpose [128, 128] blocks (two qb at once), per-partition
     scale by 1/denom, one batched store.
"""
import os
import numpy as np
import ml_dtypes

import concourse.bass as bass
import concourse.mybir as mybir
import concourse.tile as tile
from concourse import bacc
from concourse.bass_utils import run_bass_kernel_spmd
from concourse.masks import make_identity

F32 = mybir.dt.float32
BF16 = mybir.dt.bfloat16
EXP = mybir.ActivationFunctionType.Exp
NPBF16 = ml_dtypes.bfloat16

B, S, D_IN, DK, DV = 4, 4096, 1024, 64, 64
SQ = S // 2            # 2048 query rows per core
NCH = D_IN // 128      # 8 d_in chunks
NKV = S // 128         # 32 kv chunks of 128
NQB = SQ // 512        # 4 query blocks of 512
KVG = 512              # kv granule column width (4 chunks)
NG = S // KVG          # 8 kv granules

_NC_CACHE = {}


def build_attention_nc():
    nc = bacc.Bacc()

    qt_ext = nc.declare_dram_parameter("qt", [D_IN, SQ], BF16, isOutput=False)
    kt_ext = nc.declare_dram_parameter("kt", [D_IN, S], BF16, isOutput=False)
    vt_ext = nc.declare_dram_parameter("vt", [D_IN, S], BF16, isOutput=False)
    wq_ext = nc.declare_dram_parameter("wq", [D_IN, DK], F32, isOutput=False)
    wk_ext = nc.declare_dram_parameter("wk", [D_IN, DK], F32, isOutput=False)
    wv_ext = nc.declare_dram_parameter("wv", [D_IN, DV], F32, isOutput=False)
    bq_ext = nc.declare_dram_parameter("bq", [DK], F32, isOutput=False)
    bk_ext = nc.declare_dram_parameter("bk", [DK], F32, isOutput=False)
    bv_ext = nc.declare_dram_parameter("bv", [DV], F32, isOutput=False)
    out_ext = nc.declare_dram_parameter("out", [SQ, DV], F32, isOutput=True)

    qt_g = qt_ext.rearrange("(c p) s -> p c s", p=128)
    kt_g = kt_ext.rearrange("(c p) s -> p c s", p=128)
    vt_g = vt_ext.rearrange("(c p) s -> p c s", p=128)

    with tile.TileContext(nc) as tc:
        with (
            tc.tile_pool(name="sg", bufs=1) as sg,
            tc.tile_pool(name="src", bufs=4) as srcp,
            tc.tile_pool(name="exp", bufs=4) as expp,
            tc.tile_pool(name="fin", bufs=2) as fin,
            tc.tile_pool(name="pp", bufs=2, space="PSUM") as pp,
            tc.tile_pool(name="sc", bufs=2, space="PSUM") as scp,
            tc.tile_pool(name="ot", bufs=2, space="PSUM") as otp,
        ):
            # ---- constants
            identb = sg.tile([128, 128], BF16)
            make_identity(nc, identb[:, :])
            identf = sg.tile([128, 128], F32)
            make_identity(nc, identf[:, :])
            ones = sg.tile([128, 1], BF16)
            nc.vector.memset(ones[:, :], 1.0)

            # weights: bf16, Wq/Wk column-duplicated for row-tile packing
            Wq = sg.tile([128, NCH, 128], BF16)
            Wk = sg.tile([128, NCH, 128], BF16)
            Wv = sg.tile([128, NCH, DV], BF16)
            for W, ext in ((Wq, wq_ext), (Wk, wk_ext)):
                for h in range(2):
                    nc.gpsimd.dma_start(
                        out=W[:, :, 64 * h : 64 * h + 64],
                        in_=ext.rearrange("(c p) n -> p c n", p=128),
                    )
            nc.gpsimd.dma_start(
                out=Wv[:, :, :], in_=wv_ext.rearrange("(c p) n -> p c n", p=128)
            )
            bqd = sg.tile([128, 1], F32)
            bkd = sg.tile([128, 1], F32)
            bvd = sg.tile([64, 1], F32)
            for h in range(2):
                nc.sync.dma_start(out=bqd[64 * h : 64 * h + 64, :], in_=bq_ext[:].unsqueeze(-1))
                nc.sync.dma_start(out=bkd[64 * h : 64 * h + 64, :], in_=bk_ext[:].unsqueeze(-1))
            nc.sync.dma_start(out=bvd[:, :], in_=bv_ext[:].unsqueeze(-1))

            # projected tensors
            qTd = sg.tile([128, SQ], BF16)   # rows 0-63 = qT, 64-127 = copy
            kTd = sg.tile([128, S], BF16)    # rows 0-63 = kT, 64-127 = copy
            vT = sg.tile([64, S], BF16)      # [dv, kv]
            v1 = sg.tile([128, NKV, DV], BF16)  # v natural per chunk
            dacc = sg.tile([128, NQB, 512], BF16)  # partial softmax denominators

            # prime PE clock and keep HAM warm while first loads land
            prime_ps = pp.tile([128, 128], BF16, tag="pp")
            for _ in range(16):
                nc.tensor.transpose(prime_ps[:, :], identb[:, :], identb[:, :])

            # PV accumulators: one bank per qb pair, col-tiled M=64 each
            otAB = otp.tile([128, 512], F32, tag="ot", name="otAB")
            otCD = otp.tile([128, 512], F32, tag="ot", name="otCD")

            # ---- all input load DMAs, issued in streaming order on sync
            src_tiles = {}

            def load(kind, idx):
                t = srcp.tile([128, NCH, 1024], BF16, tag="src", name=f"src_{kind}{idx}")
                if kind == "q":
                    nc.sync.dma_start(
                        out=t[:, :, :], in_=qt_g[:, :, 1024 * idx : 1024 * idx + 1024]
                    )
                else:
                    g = kt_g if kind == "k" else vt_g
                    nc.sync.dma_start(
                        out=t[:, :, 0:KVG], in_=g[:, :, KVG * idx : KVG * idx + KVG]
                    )
                src_tiles[(kind, idx)] = t

            load("q", 0); load("k", 0); load("v", 0); load("q", 1)
            for g in range(1, NG):
                load("k", g); load("v", g)

            def project(kind, idx):
                """Project one granule; evict with bias into qTd/kTd/vT."""
                src = src_tiles.pop((kind, idx))
                W = {"q": Wq, "k": Wk, "v": Wv}[kind]
                ncols = 1024 if kind == "q" else KVG
                for s0 in range(0, ncols, 512):
                    col0 = (1024 if kind == "q" else KVG) * idx + s0
                    mdim = 128 if kind != "v" else 64
                    ps = pp.tile([128, 512], F32, tag="pp", name=f"pp_{kind}{idx}_{s0}")
                    for c in range(NCH):
                        nc.tensor.matmul(
                            ps[0:mdim, :],
                            W[:, c, 0:mdim],
                            src[:, c, s0 : s0 + 512],
                            start=(c == 0),
                            stop=(c == NCH - 1),
                        )
                    if kind == "q":
                        nc.vector.tensor_scalar_add(
                            qTd[:, col0 : col0 + 512], ps[:, :], bqd[:, :]
                        )
                    elif kind == "k":
                        nc.vector.tensor_scalar_add(
                            kTd[:, col0 : col0 + 512], ps[:, :], bkd[:, :]
                        )
                    else:
                        nc.vector.tensor_scalar_add(
                            vT[:, col0 : col0 + 512], ps[0:64, :], bvd[:, :]
                        )

            def vflip(c):
                """vT chunk c -> v1[:, c, :] (natural [kv, dv])."""
                ps = pp.tile([128, DV], BF16, tag="pp", name=f"vf{c}")
                nc.tensor.transpose(
                    ps[:, :], vT[:, 128 * c : 128 * (c + 1)], identb[0:64, 0:64]
                )
                nc.vector.tensor_copy(v1[:, c, :], ps[:, :])

            def attn_pair(p, interleave=None):
                """Scores+exp+denom+PV for chunk pair (2p, 2p+1), all qb."""
                exs = {}
                for qb in range(NQB):
                    sps = scp.tile([128, 1024], F32, tag="sc", name=f"sc{p}_{qb}")
                    for j in range(2):
                        c = 2 * p + j
                        lo, hi = 64 * j, 64 * j + 64
                        nc.tensor.matmul(
                            sps[:, 512 * j : 512 * j + 512],
                            kTd[lo:hi, 128 * c : 128 * (c + 1)],
                            qTd[lo:hi, 512 * qb : 512 * qb + 512],
                            start=True,
                            stop=True,
                        )
                    ex = expp.tile([128, 1024], BF16, tag="ex", name=f"ex{p}_{qb}")
                    nc.scalar.activation(out=ex[:, :], in_=sps[:, :], func=EXP, scale=0.125)
                    exs[qb] = ex
                    eng = nc.vector if qb < 2 else nc.gpsimd
                    if p == 0:
                        eng.tensor_add(dacc[:, qb, :], ex[:, 0:512], ex[:, 512:1024])
                    else:
                        eng.tensor_add(dacc[:, qb, :], dacc[:, qb, :], ex[:, 0:512])
                        eng.tensor_add(dacc[:, qb, :], dacc[:, qb, :], ex[:, 512:1024])
                    if qb == 1:
                        for j in range(2):
                            c = 2 * p + j
                            for half, qa in ((0, 0), (64, 1)):
                                nc.tensor.matmul(
                                    otAB[half : half + 64, :],
                                    v1[:, c, :],
                                    exs[qa][:, 512 * j : 512 * j + 512],
                                    start=(c == 0),
                                    stop=(c == NKV - 1),
                                )
                        if interleave is not None:
                            interleave()
                for j in range(2):
                    c = 2 * p + j
                    for half, qa in ((0, 2), (64, 3)):
                        nc.tensor.matmul(
                            otCD[half : half + 64, :],
                            v1[:, c, :],
                            exs[qa][:, 512 * j : 512 * j + 512],
                            start=(c == 0),
                            stop=(c == NKV - 1),
                        )

            # ---- prologue projections
            project("q", 0)
            project("k", 0)
            project("v", 0)
            for c in range(4):
                vflip(c)
            project("q", 1)

            # ---- main streaming loop over kv granules
            for g in range(NG):
                work = []
                if g + 1 < NG:
                    work = [
                        lambda g=g: project("k", g + 1),
                        lambda g=g: project("v", g + 1),
                        lambda g=g: [vflip(4 * (g + 1) + i) for i in range(4)],
                    ]
                it = iter(work)

                def step(it=it):
                    nxt = next(it, None)
                    if nxt is not None:
                        nxt()

                attn_pair(2 * g, interleave=step)
                step()
                attn_pair(2 * g + 1, interleave=step)
                step()

            # ---- epilogue: denominators, normalize, store
            dn = pp.tile([128, 16], F32, tag="pp", name="dn")
            for qb in range(NQB):
                for t in range(4):
                    nc.tensor.matmul(
                        dn[:, 4 * qb + t : 4 * qb + t + 1],
                        dacc[:, qb, 128 * t : 128 * t + 128],
                        ones[:, :],
                        start=True,
                        stop=True,
                    )
            rd = fin.tile([128, 16], F32, tag="rd")
            nc.vector.reciprocal(rd[:, :], dn[:, :])

            osbAB = fin.tile([128, 512], F32, tag="osb", name="osbAB")
            osbCD = fin.tile([128, 512], F32, tag="osb", name="osbCD")
            nc.vector.tensor_copy(osbAB[:, :], otAB[:, :])
            nc.vector.tensor_copy(osbCD[:, :], otCD[:, :])

            stage = sg.tile([128, 16, DV], F32)
            for pair, osb in ((0, osbAB), (1, osbCD)):
                for t in range(4):
                    tp = pp.tile([128, 128], F32, tag="pp", name=f"tp{pair}_{t}")
                    nc.tensor.transpose(
                        tp[:, :], osb[:, 128 * t : 128 * t + 128], identf[:, :]
                    )
                    for h in range(2):
                        qb = 2 * pair + h
                        nc.vector.tensor_scalar_mul(
                            stage[:, 4 * qb + t, :],
                            tp[:, 64 * h : 64 * h + 64],
                            rd[:, 4 * qb + t : 4 * qb + t + 1],
                        )
            nc.sync.dma_start(
                out=out_ext.rearrange("(b p) n -> p b n", p=128), in_=stage[:, :, :]
            )

    nc.compile()
    return nc


def _get_nc():
    if "nc" not in _NC_CACHE:
        _NC_CACHE["nc"] = build_attention_nc()
    return _NC_CACHE["nc"]


def kernel(query, key, value, Wq, bq, Wk, bk, Wv, bv):
    query = np.asarray(query, dtype=np.float32)
    key = np.asarray(key, dtype=np.float32)
    value = np.asarray(value, dtype=np.float32)
    wq = np.ascontiguousarray(np.asarray(Wq, np.float32))
    wk = np.ascontiguousarray(np.asarray(Wk, np.float32))
    wv = np.ascontiguousarray(np.asarray(Wv, np.float32))
    bq_ = np.ascontiguousarray(np.asarray(bq, np.float32))
    bk_ = np.ascontiguousarray(np.asarray(bk, np.float32))
    bv_ = np.ascontiguousarray(np.asarray(bv, np.float32))

    in_maps = []
    for b in range(B):
        ktb = np.ascontiguousarray(key[b].astype(NPBF16).T)
        vtb = np.ascontiguousarray(value[b].astype(NPBF16).T)
        for h in range(2):
            qtb = np.ascontiguousarray(
                query[b, h * SQ : (h + 1) * SQ].astype(NPBF16).T
            )
            in_maps.append(
                {
                    "qt": qtb, "kt": ktb, "vt": vtb,
                    "wq": wq, "wk": wk, "wv": wv,
                    "bq": bq_, "bk": bk_, "bv": bv_,
                }
            )

    nc = _get_nc()
    trace = bool(int(os.environ.get("BASS_KERNEL_TRACE", "0")))
    res = run_bass_kernel_spmd(nc, in_maps, core_ids=list(range(8)), trace=trace)
    _NC_CACHE["last_results"] = res

    out = np.empty((B, S, DV), np.float32)
    for b in range(B):
        for h in range(2):
            out[b, h * SQ : (h + 1) * SQ] = res.results[2 * b + h]["out"]
    return out
